# revision 68
# baseline (speedup 1.0000x reference)
# kernel.py — DiscriminativeLoss on 8 TRN2 NeuronCores (Bass/Tile, SPMD).
#
# Math (matches reference):
#   counts_k = #{i: l_i = k};  S_k = sum_{i in k} x_i;  mu_k = S_k / max(c_k, 1)
#   intra = (1/K) * sum_i invc_{l_i} * relu(||x_i - mu_{l_i} + eps|| - 1.5)^2
#   inter = sum_{a != b} relu(1 - ||(mu_a + eps) - mu_b||)^2 / (K*(K-1))
#   reg   = (1/K) * sum_k ||mu_k + eps||
#   total = intra + inter + 0.001 * reg
#
# V3 design — sorted single-label tiles, fp8 pass-1, RDMA mu exchange:
#   The loss is a sum over points with one global mu, so point order and
#   core assignment are free.  The host sorts points by label and pads
#   every cluster to whole 128-point tiles with identical tile counts on
#   all 8 cores, so each tile holds points of exactly ONE compile-time
#   known cluster wt[t].  Pad points carry x = 0 and count-column 0.
#   pass 1 (fp8 stream, ~4.2 MB): segment sums need no one-hots at all —
#     tiles of one cluster contract against a constant ones column.
#     Three same-cluster tiles stack into one matmul (lhsT [128, 3*33],
#     out [99, 1] at psum column wt), tripling PE instruction efficiency;
#     the [99, 64] accumulator folds to [33, 64] with two DVE adds.
#   exchange: S^T slab [128, 33] bf16 to all 7 peers via singleton
#     remote_dma_broadcasts (XOR-relative routing), ~1.5 us total instead
#     of the 15 us AllGather constant.  The receive wait is a semaphore
#     condition patched onto the first slab-sum instruction after tile
#     scheduling (the single-core scheduling sim cannot observe remote
#     sem bumps and would otherwise deadlock or hoist the wait).
#   stats: mu, 1/c, sqrt(1/c) -> table2f [64, 33] = [eps-mu | svp-1/256].
#     The per-segment gather selects its row with lhsT = e_k (an identity
#     COLUMN slice broadcast along free — free-axis offsets are legal
#     anywhere, while partition bases must be 0/32/64) and rhs = table2f
#     broadcast across the segment's tiles.
#   pass 2 (bf16 stream, ~8.5 MB, overlaps all of the above): per 15-tile
#     PSUM bank group: one batched matmul opens the accumulation with
#     psD = I @ [x | c01/256]; per tile one broadcast matmul accumulates
#     ones[1,128]^T @ tableF[0:1, wt, :], giving psD = [x-mu+eps | svp].
#     Act squares psD -> sq bf16 (col 32 -> 1/c); DVE fold-tree reduces
#     D -> d2; sqrt/relu/weighted-sum finals per sq chunk as baseline.
import math
import numpy as np
from contextlib import ExitStack

import concourse.bass as bass
import concourse.bacc as bacc
import concourse.tile as tile
import concourse.mybir as mybir
from concourse.bass_utils import run_bass_kernel_spmd

F32 = mybir.dt.float32
BF16 = mybir.dt.bfloat16
FP8 = mybir.dt.float8e4
I16 = mybir.dt.int16

N_CORES = 8
K = 64
D = 32
P = 128
EPS = 1e-8

INTRA_MARGIN = 1.5
INTER_MARGIN2 = 1.0  # 2 * 0.5

J1 = 168     # pass-1 fp8 chunk width (tiles)
NTG = 15     # pass-2 tiles per PSUM bank group (15*33*4B = 1980 <= 2048)
J2 = 45      # pass-2 xe chunk width (3 bank groups; rolling buffers give
             # natural DMA backpressure so the exchange is never parked
             # behind a long queued backlog)
SQC = 12     # bank groups per sq chunk (fold granularity)


def _plan_layout(labels):
    """Global sort + per-cluster padded tiling shared by all cores.

    Returns (order, tiles_per_cluster, wt, tpc).  Core c takes sorted
    points [c::8]; per-core cluster counts differ by at most 1, and every
    cluster k is padded to T_k = ceil(max_core_count / 128) tiles so the
    tile->cluster map wt is identical on all cores.
    """
    l = np.asarray(labels, np.int64)
    order = np.argsort(l, kind="stable")
    counts = np.bincount(l, minlength=K)
    # strided deal gives core c count = ceil((counts - c)/8) <= ceil(counts/8)
    max_core = -(-counts // N_CORES)  # ceil div
    tk = np.maximum(1, -(-max_core // P))  # tiles per cluster
    wt = np.repeat(np.arange(K), tk).astype(np.int64)
    return order, tk, wt, int(tk.sum())


def _host_prep(features, labels, order, tk, wt, tpc):
    """Relayout per core: sorted, cluster-padded, fp8 + bf16 streams."""
    import ml_dtypes

    f_s = np.asarray(features, dtype=np.float32)[order]
    l_s = np.asarray(labels, np.int64)[order]
    n_pad = P * tpc
    tstart = np.concatenate([[0], np.cumsum(tk)])  # tile offset per cluster
    in_maps = []
    for c in range(N_CORES):
        f = f_s[c::N_CORES]
        l = l_s[c::N_CORES]
        cnt = np.bincount(l, minlength=K)
        xe = np.zeros((n_pad, D + 1), np.float32)
        pos = 0
        for k in range(K):
            o = tstart[k] * P
            xe[o : o + cnt[k], :D] = f[pos : pos + cnt[k]]
            xe[o : o + cnt[k], D] = 1.0 / 256.0
            pos += cnt[k]
        xe = xe.reshape(tpc, P, D + 1).transpose(1, 0, 2)
        in_maps.append(
            {
                "xe8": np.ascontiguousarray(xe.astype(ml_dtypes.float8_e4m3fn)),
                "xe": np.ascontiguousarray(xe.astype(ml_dtypes.bfloat16)),
                "id128": np.eye(P, dtype=ml_dtypes.bfloat16),
                "id3stack": np.vstack([np.eye(D + 1)] * 3)[:99].astype(
                    ml_dtypes.bfloat16
                ),
                "id64": np.eye(K, dtype=np.float32),
                "eyeneg": (1.0 - np.eye(K, dtype=np.float32)).astype(
                    ml_dtypes.bfloat16
                ),
            }
        )
    return in_maps


def build_program(tpc, tk, wt, sq_dve_every=3, f1_pool_every=1, psmg_bufs=6, xpool_bufs=4, x8_bufs=3, sq_bufs=3):
    """Build the SPMD Bass program.  tk: [K] tiles per cluster; wt: [tpc]
    tile->cluster map — compile-time constants baked into APs."""
    nc = bacc.Bacc(
        "TRN2", target_bir_lowering=False, debug=False, num_devices=N_CORES
    )

    xe8_d = nc.dram_tensor("xe8", [P, tpc, D + 1], FP8, kind="ExternalInput").ap()
    xe_d = nc.dram_tensor("xe", [P, tpc, D + 1], BF16, kind="ExternalInput").ap()
    id128_d = nc.dram_tensor("id128", [P, P], BF16, kind="ExternalInput").ap()
    id3stack_d = nc.dram_tensor("id3stack", [99, D + 1], BF16, kind="ExternalInput").ap()
    id64_d = nc.dram_tensor("id64", [K, K], F32, kind="ExternalInput").ap()
    eyeneg_d = nc.dram_tensor("eyeneg", [K, K], BF16, kind="ExternalInput").ap()
    out_d = nc.dram_tensor("out", [3], F32, kind="ExternalOutput").ap()

    rsemA = nc.alloc_semaphore(name="x_rsemA")
    rsemB = nc.alloc_semaphore(name="x_rsemB")
    lsem = nc.alloc_semaphore(name="x_lsem")

    # pass-1 schedule: per cluster, floor(T_k/3) triples + (T_k mod 3)
    # singles; each entry = (first_tile, n_tiles, cluster)
    p1_sched = []
    t0 = 0
    for k in range(K):
        t = t0
        left = int(tk[k])
        while left >= 3:
            p1_sched.append((t, 3, k))
            t += 3
            left -= 3
        while left:
            p1_sched.append((t, 1, k))
            t += 1
            left -= 1
        t0 += int(tk[k])

    n_chunks1 = math.ceil(tpc / J1)
    n_chunks2 = math.ceil(tpc / J2)
    n_groups = math.ceil(tpc / NTG)
    n_sqc = math.ceil(n_groups / SQC)

    with tile.TileContext(nc, num_cores=N_CORES) as tc, ExitStack() as ctx:
        singles = ctx.enter_context(tc.tile_pool(name="singles", bufs=1))
        x8pool = ctx.enter_context(tc.tile_pool(name="x8pool", bufs=x8_bufs))
        xpool = ctx.enter_context(tc.tile_pool(name="xpool", bufs=xpool_bufs))
        sqpool = ctx.enter_context(tc.tile_pool(name="sqpool", bufs=sq_bufs))
        wpool = ctx.enter_context(tc.tile_pool(name="wpool", bufs=2))
        psS = ctx.enter_context(tc.tile_pool(name="psS", bufs=1, space="PSUM"))
        psT = ctx.enter_context(tc.tile_pool(name="psT", bufs=1, space="PSUM"))
        psMg = ctx.enter_context(tc.tile_pool(name="psMg", bufs=psmg_bufs, space="PSUM"))

        # ---------- constants ----------
        onescol = singles.tile([P, 1], FP8)  # ones column (pass-1 rhs)
        nc.gpsimd.memset(onescol, 1.0)
        z99 = singles.tile([1, 99], FP8)
        nc.gpsimd.memset(z99, 0.0)
        z64 = singles.tile([1, K], FP8)
        nc.gpsimd.memset(z64, 0.0)

        # ---------- pass 1: segment sums from the fp8 stream ----------
        # psumS2 [99, 64]: three stacked 33-row sum blocks; column = cluster
        psumS2 = psS.tile([99, K], F32, tag="psa")
        nc.tensor.matmul(psumS2, z99, z64, start=True, stop=False)
        chunk_ops = [[] for _ in range(n_chunks1)]  # (local_t, ntl, k)
        for (t, ntl, k) in p1_sched:
            c_a, c_b = t // J1, (t + ntl - 1) // J1
            if c_a == c_b:
                chunk_ops[c_a].append((t - c_a * J1, ntl, k))
            else:
                for tt in range(t, t + ntl):
                    cc = tt // J1
                    chunk_ops[cc].append((tt - cc * J1, 1, k))

        for c1 in range(n_chunks1):
            j0 = c1 * J1
            jn = min(J1, tpc - j0)
            xe8 = x8pool.tile([P, J1, D + 1], FP8, tag="xe8")
            nc.sync.dma_start(out=xe8[:, :jn, :], in_=xe8_d[:, j0 : j0 + jn, :])
            for (jo, ntl, k) in chunk_ops[c1]:
                nc.tensor.matmul(
                    psumS2[0 : 33 * ntl, k : k + 1],
                    xe8[:, jo : jo + ntl, :],
                    onescol,
                    start=False,
                    stop=False,
                )
        nc.tensor.matmul(psumS2, z99, z64, start=False, stop=True)

        # constants ahead of the long bf16 stream on the SP queue
        id3stack = singles.tile([99, D + 1], BF16)
        nc.sync.dma_start(out=id3stack, in_=id3stack_d)
        id128 = singles.tile([P, P], BF16)
        nc.sync.dma_start(out=id128, in_=id128_d)
        id64 = singles.tile([K, K], F32)
        nc.sync.dma_start(out=id64, in_=id64_d)
        eyeneg = singles.tile([K, K], BF16)
        nc.sync.dma_start(out=eyeneg, in_=eyeneg_d)

        # fold the three stacked 33-row blocks AND transpose in one matmul
        cp99b = wpool.tile([99, K], BF16, tag="cp99")
        nc.scalar.copy(out=cp99b, in_=psumS2)
        psW = psT.tile([K, D + 1], F32, tag="psw")
        nc.tensor.matmul(psW, cp99b, id3stack)
        slab = singles.tile([P, D + 1], BF16)
        nc.gpsimd.memset(slab, 0.0)
        nc.scalar.copy(out=slab[:K, :], in_=psW)
        xslab = singles.tile([P, N_CORES, D + 1], BF16)
        nc.scalar.copy(out=xslab[:, 0, :], in_=slab)
        for d in range(1, N_CORES):
            rdests = [None] * 8
            rdests[d] = (0, d)
            nc.gpsimd.remote_dma_broadcast(
                out_ap=xslab[:, d, :],
                in_ap=slab,
                remote_sem=rsemA,
                local_sem=lsem,
                rdests=rdests,
            )
        nc.gpsimd.trigger_dma(count=None)

        # ---------- slab sums ----------
        s4a = wpool.tile([P, 4, D + 1], F32, tag="s4")
        first_sum_a = nc.vector.tensor_add(
            s4a, xslab[:, 0:4, :], xslab[:, 4:8, :]
        )
        s2a = wpool.tile([P, 2, D + 1], F32, tag="s2")
        nc.vector.tensor_add(s2a, s4a[:, 0:2, :], s4a[:, 2:4, :])
        Wf = singles.tile([P, D + 1], F32)  # rows 0:64 = [S_k | c_k/256]
        nc.vector.tensor_add(Wf[0:K, :], s2a[0:K, 0, :], s2a[0:K, 1, :])

        # ---------- stats (DVE-heavy to avoid Pool q7-launch latency) -----
        safec = wpool.tile([K, 1], F32, tag="safec")
        nc.vector.tensor_scalar(
            safec, Wf[:K, D : D + 1], 256.0, 1.0,
            mybir.AluOpType.mult, mybir.AluOpType.max,
        )
        invc = wpool.tile([K, 1], F32, tag="invc")
        nc.vector.reciprocal(invc, safec)
        svp = wpool.tile([K, 1], F32, tag="svp")
        nc.scalar.activation(
            out=svp, in_=invc, func=mybir.ActivationFunctionType.Sqrt
        )
        mu = wpool.tile([K, D], F32, tag="mu")
        nc.vector.tensor_mul(mu, Wf[:K, :D], invc.to_broadcast((K, D)))
        # table2f [64, 33] bf16 = [eps - mu | svp - 1/256]
        table2f = singles.tile([K, D + 1], BF16)
        nc.scalar.activation(
            out=table2f[:, :D], in_=mu,
            func=mybir.ActivationFunctionType.Copy, bias=EPS, scale=-1.0,
        )
        nc.scalar.activation(
            out=table2f[:, D : D + 1], in_=svp,
            func=mybir.ActivationFunctionType.Copy, bias=-1.0 / 256.0,
        )

        # ---------- inter + reg losses (off critical path) ----------
        mup = wpool.tile([K, D], F32, tag="mup")
        nc.scalar.activation(
            out=mup, in_=mu, func=mybir.ActivationFunctionType.Copy, bias=EPS
        )
        qsc = wpool.tile([K, D], F32, tag="qsc")
        nc.gpsimd.tensor_mul(qsc, mu, mu)
        q = wpool.tile([K, 1], F32, tag="q")
        nc.vector.tensor_reduce(
            out=q, in_=qsc, axis=mybir.AxisListType.X, op=mybir.AluOpType.add
        )
        qpsc = wpool.tile([K, D], F32, tag="qpsc")
        nc.gpsimd.tensor_mul(qpsc, mup, mup)
        qp = wpool.tile([K, 1], F32, tag="qp")
        nc.vector.tensor_reduce(
            out=qp, in_=qpsc, axis=mybir.AxisListType.X, op=mybir.AluOpType.add
        )
        ab = wpool.tile([K, D + 2], F32, tag="ab")
        nc.scalar.mul(out=ab[:, :D], in_=mup, mul=-2.0)
        nc.scalar.copy(out=ab[:, D : D + 1], in_=qp)
        nc.gpsimd.memset(ab[:, D + 1 : D + 2], 1.0)
        bb = wpool.tile([K, D + 2], F32, tag="bb")
        nc.scalar.copy(out=bb[:, :D], in_=mu)
        nc.gpsimd.memset(bb[:, D : D + 1], 1.0)
        nc.scalar.copy(out=bb[:, D + 1 : D + 2], in_=q)
        psTa = psT.tile([D + 2, K], F32, tag="psw")
        nc.tensor.transpose(psTa, ab, id64)
        atp = wpool.tile([D + 2, K], F32, tag="atp")
        nc.scalar.copy(out=atp, in_=psTa)
        psTb = psT.tile([D + 2, K], F32, tag="psw")
        nc.tensor.transpose(psTb, bb, id64)
        btp = wpool.tile([D + 2, K], F32, tag="btp")
        nc.scalar.copy(out=btp, in_=psTb)
        psPD = psT.tile([K, K], F32, tag="psw")
        nc.tensor.matmul(psPD, atp, btp)
        pdc = wpool.tile([K, K], F32, tag="pdc")
        nc.vector.tensor_scalar_max(pdc, psPD, 0.0)
        pdist = wpool.tile([K, K], F32, tag="pdist")
        nc.scalar.activation(
            out=pdist, in_=pdc, func=mybir.ActivationFunctionType.Sqrt
        )
        hingeI = wpool.tile([K, K], F32, tag="hingeI")
        nc.scalar.activation(
            out=hingeI, in_=pdist, func=mybir.ActivationFunctionType.Relu,
            bias=float(INTER_MARGIN2), scale=-1.0,
        )
        hm = wpool.tile([K, K], F32, tag="hm")
        nc.gpsimd.tensor_mul(hm, hingeI, eyeneg)
        hm2 = wpool.tile([K, K], F32, tag="hm2")
        nc.gpsimd.tensor_mul(hm2, hm, hm)
        interp = wpool.tile([K, 1], F32, tag="interp")
        nc.vector.tensor_reduce(
            out=interp, in_=hm2, axis=mybir.AxisListType.X,
            op=mybir.AluOpType.add,
        )
        sqp = wpool.tile([K, 1], F32, tag="sqp")
        nc.scalar.activation(
            out=sqp, in_=qp, func=mybir.ActivationFunctionType.Sqrt
        )
        cat2 = wpool.tile([K, 2], F32, tag="cat2")
        nc.scalar.copy(out=cat2[:, 0:1], in_=interp)
        nc.scalar.copy(out=cat2[:, 1:2], in_=sqp)
        ones64 = singles.tile([K, 1], F32)
        nc.gpsimd.memset(ones64, 1.0)
        psIR = psT.tile([1, 2], F32, tag="psw")
        nc.tensor.matmul(psIR, ones64, cat2)
        ir = wpool.tile([1, 2], F32, tag="ir")
        nc.scalar.copy(out=ir, in_=psIR)



        # ---------- pass 2 ----------
        d2all = singles.tile([P, tpc], F32)
        invc_all = singles.tile([P, tpc], BF16)
        hh = singles.tile([P, tpc], F32)
        hhw = singles.tile([P, tpc], F32)
        rsacc = singles.tile([P, n_sqc], F32)
        margneg = singles.tile([P, 1], F32)
        nc.gpsimd.memset(margneg, -float(INTRA_MARGIN))

        for sc in range(n_sqc):
            g0 = sc * SQC
            gn = min(SQC, n_groups - g0)
            cbase = g0 * NTG
            ctn = min(gn * NTG, tpc - cbase)
            sq = sqpool.tile([P, SQC * NTG, D + 1], BF16, tag="sq")
            for gg in range(gn):
                g = g0 + gg
                tb = g * NTG
                ntg = min(NTG, tpc - tb)
                # one xe chunk (J2 tiles) feeds J2/NTG bank groups
                if gg % (J2 // NTG) == 0:
                    jb = tb
                    jn = min(J2, tpc - jb)
                    xb = xpool.tile([P, J2, D + 1], BF16, tag="xe")
                    nc.sync.dma_start(
                        out=xb[:, :jn, :], in_=xe_d[:, jb : jb + jn, :]
                    )
                lb = (gg % (J2 // NTG)) * NTG  # offset inside chunk buffer
                psD = psMg.tile([P, NTG, D + 1], F32, tag="psd")
                segs = []
                s0 = 0
                for t in range(1, ntg):
                    if wt[tb + t] != wt[tb + s0]:
                        segs.append((s0, t))
                        s0 = t
                segs.append((s0, ntg))
                for (sa, sb_) in segs:
                    ns = sb_ - sa
                    ek = id128[0:K, wt[tb + sa], None].to_broadcast((K, P))
                    row = table2f[:, None, :].to_broadcast((K, ns, D + 1))
                    # x-part first: no mu dependency, fills the pre-mu
                    # window; the gather closes the group
                    nc.tensor.matmul(
                        psD[:, sa:sb_, :], id128,
                        xb[:, lb + sa : lb + sb_, :],
                        start=True, stop=False,
                    )
                    nc.tensor.matmul(
                        psD[:, sa:sb_, :], ek, row,
                        start=False, stop=True,
                    )
                o0 = gg * NTG
                if sq_dve_every and g % sq_dve_every == sq_dve_every - 1:
                    cpy = sqpool.tile([P, NTG, D + 1], BF16, tag="cpy")
                    nc.vector.tensor_scalar_add(
                        cpy[:, :ntg, :], psD[:, :ntg, :], 0.0
                    )
                    nc.vector.tensor_mul(
                        sq[:, o0 : o0 + ntg, :],
                        cpy[:, :ntg, :], cpy[:, :ntg, :],
                    )
                else:
                    nc.scalar.activation(
                        out=sq[:, o0 : o0 + ntg, :], in_=psD[:, :ntg, :],
                        func=mybir.ActivationFunctionType.Square,
                    )
            # fold-tree reduce over D (bf16 2x) + invc extract
            with nc.allow_low_precision(reason="bf16 partial sums of d2"):
                f1 = sqpool.tile([P, SQC * NTG, 16], BF16, tag="f1")
                f1eng = nc.gpsimd if (
                    f1_pool_every and sc % f1_pool_every == f1_pool_every - 1
                ) else nc.vector
                f1eng.tensor_add(
                    f1[:, :ctn, :], sq[:, :ctn, 0:16], sq[:, :ctn, 16:32]
                )
                f2 = sqpool.tile([P, SQC * NTG, 8], BF16, tag="f2")
                nc.vector.tensor_add(
                    f2[:, :ctn, :], f1[:, :ctn, 0:8], f1[:, :ctn, 8:16]
                )
                f3 = sqpool.tile([P, SQC * NTG, 4], BF16, tag="f3")
                nc.vector.tensor_add(
                    f3[:, :ctn, :], f2[:, :ctn, 0:4], f2[:, :ctn, 4:8]
                )
                f4 = sqpool.tile([P, SQC * NTG, 2], BF16, tag="f4")
                nc.vector.tensor_add(
                    f4[:, :ctn, :], f3[:, :ctn, 0:2], f3[:, :ctn, 2:4]
                )
            nc.vector.tensor_tensor(
                d2all[:, cbase : cbase + ctn],
                f4[:, :ctn, 0], f4[:, :ctn, 1], mybir.AluOpType.add,
            )
            nc.gpsimd.tensor_scalar_add(
                invc_all[:, cbase : cbase + ctn], sq[:, :ctn, D], 0.0
            )
            # per-sq-chunk finals: dist, hinge, weighted sums
            dsl = d2all[:, cbase : cbase + ctn]
            nc.scalar.activation(
                out=dsl, in_=dsl, func=mybir.ActivationFunctionType.Sqrt
            )
            nc.scalar.activation(
                out=dsl, in_=dsl,
                func=mybir.ActivationFunctionType.Relu, bias=margneg,
            )
            hsl = hh[:, cbase : cbase + ctn]
            nc.scalar.activation(
                out=hsl, in_=dsl, func=mybir.ActivationFunctionType.Square
            )
            wsl = hhw[:, cbase : cbase + ctn]
            nc.vector.tensor_mul(wsl, hsl, invc_all[:, cbase : cbase + ctn])
            nc.vector.tensor_reduce(
                out=rsacc[:, sc : sc + 1], in_=wsl,
                axis=mybir.AxisListType.X, op=mybir.AluOpType.add,
            )

        # ---------- finals ----------
        rowsum = singles.tile([P, 1], F32)
        nc.vector.tensor_reduce(
            out=rowsum, in_=rsacc, axis=mybir.AxisListType.X,
            op=mybir.AluOpType.add,
        )
        ones128 = singles.tile([P, 1], F32)
        nc.gpsimd.memset(ones128, 1.0)
        psL = psT.tile([1, 1], F32, tag="psw")
        nc.tensor.matmul(psL, rowsum, ones128)
        tot = wpool.tile([1, 3], F32, tag="tot")
        nc.scalar.copy(out=tot[:, 0:1], in_=psL)
        nc.scalar.copy(out=tot[:, 1:3], in_=ir)
        nc.sync.dma_start(out=out_d, in_=tot[0:1, :])

    # each of the 7 peers bumps rsemA by 16//8 = 2 on arrival
    first_sum_a.ins.sync_info.on_wait.append(
        mybir.SyncWait(
            sync_type="semaphore",
            id=rsemA.num,
            ant_name=rsemA.name,
            wait_mode="sem-ge-imm",
            wait_value=2 * (N_CORES - 1),
            wait_reg=None,
        )
    )
    nc.compile()
    return nc


_NC_CACHE = {}


def kernel(features, labels, num_clusters):
    features = np.asarray(features)
    labels = np.asarray(labels)
    order, tk, wt, tpc = _plan_layout(labels)
    in_maps = _host_prep(features, labels, order, tk, wt, tpc)
    key = (tpc, wt.tobytes())
    if key not in _NC_CACHE:
        _NC_CACHE[key] = build_program(
            tpc, [int(v) for v in tk], [int(v) for v in wt]
        )
    nc = _NC_CACHE[key]
    res = run_bass_kernel_spmd(nc, in_maps, list(range(N_CORES)))
    intra_sum = sum(float(res.results[c]["out"][0]) for c in range(N_CORES))
    inter_sum = float(res.results[0]["out"][1])
    reg_sum = float(res.results[0]["out"][2])
    total = (
        intra_sum / K
        + inter_sum / (K * (K - 1))
        + 0.001 * reg_sum / K
    )
    return np.float32(total)
